# revision 10
# baseline (speedup 1.0000x reference)
"""MultiHeadAttention Trainium2 Bass kernel.

Model: B=2, S=2048, D_MODEL=1024, H=16 heads, Dh=64.
  q/k/v = x @ W.T + b ; scores = (q k^T)/8 masked-softmax ; out = w @ v ; y = out @ Wy.T + by

Sharding: (batch x sequence) data parallel over 8 cores. Core c handles
batch b = c // 4 and query rows [q0, q0+512) with q0 = (c % 4) * 512.
The K/V projections are token-sharded across the 4 cores of a batch group
(each core projects only its own 512-token chunk) and the results are
AllGathered inside the group, so no projection work is replicated. The
attention and output projection are computed only for the core's query
slice; the final output is a pure concatenation of per-core slices.

Implementation notes:
  - All matmul operands are bf16 (host pre-converts weights/activations);
    PSUM accumulation is fp32. End-to-end max-rel-err ~1.3e-2 vs the 2e-2
    budget. The hardware is bound by the PE instruction stream (~0.7us per
    matmul on this part), so the kernel minimizes matmul count: 768 total.
  - Scores are computed transposed, scoresT[k, q]; heads are processed in
    pairs (2m, 2m+1) on partition halves, the two 64-contraction score
    matmuls issued to disjoint PE row groups via tile_position.
  - Softmax: no max-subtraction (scores are O(6)); exp on ACT straight from
    PSUM, then multiply by the {0,1} int8 mask in-place (alternating
    DVE/GpSimd). The denominator rides as an extra all-ones column
    interleaved in the SBUF-resident V tiles ([V_h(64) | ones(1)] blocks);
    the divide uses reciprocal + GpSimd partition_broadcast.
"""

import numpy as np

import concourse.bass as bass
import concourse.mybir as mybir
import concourse.tile as tile
from concourse import bacc
from concourse.bass_utils import run_bass_kernel_spmd

F32 = mybir.dt.float32
BF16 = mybir.dt.bfloat16
I8 = mybir.dt.int8

B, S, D, H, DH = 2, 2048, 1024, 16, 64
QS = 512          # query rows / projection-token chunk per core
P = 128
KO = D // P       # 8 contraction tiles for the projections
NKT = S // P      # 16 key tiles
E = DH + 1        # V block width incl. ones column
NG = 4            # cores per batch group

_CACHE = {}


def build_program():
    nc = bacc.Bacc("TRN2", target_bir_lowering=False, debug=False, num_devices=8)

    qT = nc.dram_tensor("qT", [D, QS], BF16, kind="ExternalInput")    # queries[b].T q-slice
    kT = nc.dram_tensor("kT", [D, QS], BF16, kind="ExternalInput")    # keys[b].T token chunk
    vT = nc.dram_tensor("vT", [D, QS], BF16, kind="ExternalInput")    # values[b].T token chunk
    maskb = nc.dram_tensor("maskb", [H, P, NKT * QS], I8, kind="ExternalInput")
    WqT = nc.dram_tensor("WqT", [D, D], BF16, kind="ExternalInput")   # (Wq/8).T
    WkT = nc.dram_tensor("WkT", [D, D], BF16, kind="ExternalInput")
    WvT = nc.dram_tensor("WvT", [D, D], BF16, kind="ExternalInput")
    WyT = nc.dram_tensor("WyT", [D, D], BF16, kind="ExternalInput")
    bq = nc.dram_tensor("bq", [P, KO], F32, kind="ExternalInput")     # ((bq+bq2)/8) as [p, m]
    bk = nc.dram_tensor("bk", [P, KO], F32, kind="ExternalInput")
    bv = nc.dram_tensor("bv", [1, D], BF16, kind="ExternalInput")
    by = nc.dram_tensor("by", [1, D], BF16, kind="ExternalInput")
    y = nc.dram_tensor("y", [QS, D], F32, kind="ExternalOutput")

    qT_r = qT.rearrange("(ko p) q -> p ko q", p=P)
    kT_r = kT.rearrange("(ko p) s -> p ko s", p=P)
    vT_r = vT.rearrange("(ko p) s -> p ko s", p=P)
    WqT_r = WqT.rearrange("(ko p) m -> p ko m", p=P)
    WkT_r = WkT.rearrange("(ko p) m -> p ko m", p=P)
    WvT_r = WvT.rearrange("(ko p) m -> p ko m", p=P)
    WyT_r = WyT.rearrange("(ko p) m -> p ko m", p=P)

    def bcast_dram(ap, parts):
        return bass.AP(tensor=ap.tensor, offset=ap.offset, ap=[[0, parts]] + list(ap.ap[1:]))

    IDENT = mybir.ActivationFunctionType.Identity
    EXP = mybir.ActivationFunctionType.Exp
    ADD = mybir.AluOpType.add
    MULT = mybir.AluOpType.mult
    BYPASS = mybir.AluOpType.bypass
    GROUPS = [[0, 1, 2, 3], [4, 5, 6, 7]]

    with tile.TileContext(nc) as tc:
        with (
            tc.tile_pool(name="persist", bufs=1) as persist,
            tc.tile_pool(name="w", bufs=3) as wpool,
            tc.tile_pool(name="kcol", bufs=1) as kcolp,
            tc.tile_pool(name="vtc", bufs=1) as vtcp,
            tc.tile_pool(name="qin", bufs=1) as qinp,
            tc.tile_pool(name="locout", bufs=1) as locoutp,
            tc.tile_pool(name="maskp", bufs=2) as maskp,
            tc.tile_pool(name="eTr", bufs=2) as eTrp,
            tc.tile_pool(name="rec", bufs=2) as recp,
            tc.tile_pool(name="bc", bufs=2) as bcp,
            tc.tile_pool(name="yout", bufs=2) as youtp,
            tc.tile_pool(name="dram", bufs=1, space="DRAM") as dramp,
            tc.tile_pool(name="psA", bufs=2, space="PSUM") as psA,
            tc.tile_pool(name="psS", bufs=2, space="PSUM") as psS,
            tc.tile_pool(name="psT", bufs=2, space="PSUM") as psT,
        ):
            # ---- persistent SBUF ----
            bk_sb = persist.tile([P, KO], F32, tag="bk")
            bq_sb = persist.tile([P, KO], F32, tag="bq")
            bv_bc = persist.tile([P, D], BF16, tag="bv")
            by_bc = persist.tile([P, D], BF16, tag="by")
            nc.sync.dma_start(out=bk_sb, in_=bk[:])
            nc.sync.dma_start(out=bq_sb, in_=bq[:])
            nc.sync.dma_start(out=bv_bc, in_=bcast_dram(bv[:], P))
            nc.sync.dma_start(out=by_bc, in_=bcast_dram(by[:], P))

            KT_m = [persist.tile([P, S], BF16, tag=f"ktm{m}", name=f"KT_m{m}")
                    for m in range(KO)]
            QT_m = [persist.tile([P, QS], BF16, tag=f"qtm{m}", name=f"QT_m{m}")
                    for m in range(KO)]
            VS = [persist.tile([P, H * E], BF16, tag=f"vs{st}", name=f"VS{st}")
                  for st in range(NKT)]
            AT = [persist.tile([P, QS], BF16, tag=f"at{m}", name=f"AT{m}")
                  for m in range(KO)]

            # DRAM bounce buffers for the projection AllGathers
            ktloc0 = dramp.tile([P, KO // 2, QS], BF16, tag="ktloc0", name="ktloc0")
            ktloc1 = dramp.tile([P, KO // 2, QS], BF16, tag="ktloc1", name="ktloc1")
            ktgath0 = dramp.tile([NG, P, KO // 2, QS], BF16, tag="ktgath0", name="ktgath0")
            ktgath1 = dramp.tile([NG, P, KO // 2, QS], BF16, tag="ktgath1", name="ktgath1")
            vsloc = dramp.tile([P, NG, H * E], BF16, tag="vsloc", name="vsloc")
            vsgath = dramp.tile([NG, P, NG, H * E], BF16, tag="vsgath", name="vsgath")

            wk = wpool.tile([P, KO, D], BF16, tag="w")
            nc.sync.dma_start(out=wk, in_=WkT_r[:])

            # ---- phase K (local chunk): ktl[p, m, q] = Wk @ keysT_chunk + bk ----
            kcol = kcolp.tile([P, KO, QS], BF16, tag="kcol")
            nc.sync.dma_start(out=kcol, in_=kT_r[:])
            ktl = locoutp.tile([P, KO, QS], BF16, tag="loc", name="ktl")
            wq = wpool.tile([P, KO, D], BF16, tag="w")
            nc.sync.dma_start(out=wq, in_=WqT_r[:])
            wv = wpool.tile([P, KO, D], BF16, tag="w")
            nc.sync.dma_start(out=wv, in_=WvT_r[:])
            for half_m in range(2):
                for ml in range(KO // 2):
                    m = half_m * (KO // 2) + ml
                    ps = psA.tile([P, 512], F32, tag="proj", name="ps")
                    for ko in range(KO):
                        nc.tensor.matmul(
                            ps[:], wk[:, ko, m * P:(m + 1) * P], kcol[:, ko, :],
                            start=(ko == 0), stop=(ko == KO - 1))
                    nc.vector.tensor_scalar_add(
                        ktl[:, m, :], ps[:], bk_sb[:, m:m + 1])
                ktloc = ktloc0 if half_m == 0 else ktloc1
                ktgath = ktgath0 if half_m == 0 else ktgath1
                nc.sync.dma_start(
                    out=ktloc, in_=ktl[:, half_m * (KO // 2):(half_m + 1) * (KO // 2), :])
                nc.gpsimd.collective_compute(
                    "AllGather", BYPASS, replica_groups=GROUPS,
                    ins=[ktloc.opt()], outs=[ktgath.opt()])
                for ml in range(KO // 2):
                    m = half_m * (KO // 2) + ml
                    nc.sync.dma_start(
                        out=KT_m[m].rearrange("p (r q) -> p r q", r=NG),
                        in_=ktgath[:, :, ml, :].rearrange("r p q -> p r q"))

            # ---- phase V (local chunk) into vsl with interleaved ones ----
            vtc = vtcp.tile([P, KO, QS], BF16, tag="vtc")
            nc.sync.dma_start(out=vtc, in_=vT_r[:])
            vsl = locoutp.tile([P, NG, H * E], BF16, tag="loc", name="vsl")
            vsl_r = vsl.rearrange("p st (h e) -> p st h e", e=E)
            for stl in range(NG):
                for half in range(2):
                    ps = psA.tile([P, 512], F32, tag="proj", name="ps")
                    for ko in range(KO):
                        nc.tensor.matmul(
                            ps[:], vtc[:, ko, stl * P:(stl + 1) * P],
                            wv[:, ko, half * 512:(half + 1) * 512],
                            start=(ko == 0), stop=(ko == KO - 1))
                    nc.vector.tensor_tensor(
                        vsl_r[:, stl, half * 8:(half + 1) * 8, 0:DH],
                        ps.rearrange("p (h d) -> p h d", d=DH),
                        bv_bc[:, half * 512:(half + 1) * 512].rearrange(
                            "p (h d) -> p h d", d=DH),
                        ADD)
            nc.vector.memset(vsl_r[:, :, :, DH:E], 1.0)
            nc.sync.dma_start(out=vsloc, in_=vsl)
            nc.gpsimd.collective_compute(
                "AllGather", BYPASS, replica_groups=GROUPS,
                ins=[vsloc.opt()], outs=[vsgath.opt()])
            for st in range(NKT):
                nc.sync.dma_start(
                    out=VS[st], in_=vsgath[st // NG, :, st % NG, :])

            # ---- phase Q: QT_m[m][dout_p, q] = (Wq/8) @ queries[b].T + bq/8 ----
            qin = qinp.tile([P, KO, QS], BF16, tag="qin")
            nc.sync.dma_start(out=qin, in_=qT_r[:])
            for m in range(KO):
                ps = psA.tile([P, 512], F32, tag="proj", name="ps")
                for ko in range(KO):
                    nc.tensor.matmul(
                        ps[:], wq[:, ko, m * P:(m + 1) * P], qin[:, ko, :],
                        start=(ko == 0), stop=(ko == KO - 1))
                nc.vector.tensor_scalar_add(QT_m[m][:], ps[:], bq_sb[:, m:m + 1])

            # ---- attention pair hm: heads (2hm, 2hm+1) on partition halves ----
            def attn_pair(hm):
                hA, hB = 2 * hm, 2 * hm + 1
                mks = []
                for mh in range(2):
                    mk = maskp.tile([P, 2, NKT * QS // 2], I8, tag="mask", name="mk")
                    nc.sync.dma_start(
                        out=mk[:, 0, :],
                        in_=maskb[hA, :, mh * 4096:(mh + 1) * 4096])
                    nc.sync.dma_start(
                        out=mk[:, 1, :],
                        in_=maskb[hB, :, mh * 4096:(mh + 1) * 4096])
                    mks.append(mk)
                pattA = psT.tile([P, QS], F32, tag="patt", name="pattA")
                pattB = psT.tile([P, QS], F32, tag="patt", name="pattB")
                for kt in range(NKT):
                    pscr = psS.tile([P, 2 * QS], F32, tag="scores", name="pscr")
                    nc.tensor.matmul(
                        pscr[:, 0:QS], KT_m[hm][0:DH, kt * P:(kt + 1) * P],
                        QT_m[hm][0:DH, :], start=True, stop=True,
                        tile_position=(0, 0))
                    nc.tensor.matmul(
                        pscr[:, QS:2 * QS], KT_m[hm][DH:P, kt * P:(kt + 1) * P],
                        QT_m[hm][DH:P, :], start=True, stop=True,
                        tile_position=(64, 0))
                    eT = eTrp.tile([P, 2 * QS], BF16, tag="eTr", name="eT")
                    nc.scalar.activation(out=eT[:], in_=pscr[:], func=EXP)
                    mslice = mks[kt // 8][:, :, (kt % 8) * QS:(kt % 8 + 1) * QS]
                    ev = eT.rearrange("p (h q) -> p h q", q=QS)
                    eng = nc.vector if kt % 2 == 0 else nc.gpsimd
                    eng.tensor_tensor(ev[:], ev[:], mslice, MULT)
                    vs_r = VS[kt].rearrange("p (h e) -> p h e", e=E)
                    nc.tensor.matmul(
                        pattA[0:E, :], vs_r[:, hA, :], eT[:, 0:QS],
                        start=(kt == 0), stop=(kt == NKT - 1))
                    nc.tensor.matmul(
                        pattB[0:E, :], vs_r[:, hB, :], eT[:, QS:2 * QS],
                        start=(kt == 0), stop=(kt == NKT - 1))
                for hp, patt in ((0, pattA), (1, pattB)):
                    rec = recp.tile([1, QS], F32, tag="rec", name="rec")
                    nc.vector.reciprocal(out=rec[:], in_=patt[DH:E, :])
                    bc = bcp.tile([DH, QS], F32, tag="bc", name="bc")
                    nc.gpsimd.partition_broadcast(bc[:], rec[:])
                    nc.vector.tensor_tensor(
                        AT[hm][hp * DH:(hp + 1) * DH, :], patt[0:DH, :], bc[:],
                        MULT)

            for hm in range(KO):
                attn_pair(hm)

            # ---- phase Y: y = merged @ Wy.T + by ----
            wy = wpool.tile([P, KO, D], BF16, tag="w")
            nc.sync.dma_start(out=wy, in_=WyT_r[:])
            for qt in range(4):
                for half in range(2):
                    ps = psA.tile([P, 512], F32, tag="proj", name="ps")
                    for hm in range(KO):
                        nc.tensor.matmul(
                            ps[:], AT[hm][:, qt * P:(qt + 1) * P],
                            wy[:, hm, half * 512:(half + 1) * 512],
                            start=(hm == 0), stop=(hm == KO - 1))
                    yo = youtp.tile([P, 512], F32, tag="yo", name="yo")
                    nc.vector.tensor_tensor(
                        yo[:], ps[:], by_bc[:, half * 512:(half + 1) * 512], ADD)
                    nc.sync.dma_start(
                        out=y[qt * P:(qt + 1) * P, half * 512:(half + 1) * 512],
                        in_=yo[:])

    nc.compile()
    return nc


def prep_inputs(queries, keys, values, mask, Wq, bq, Wk, bk, Wv, bv, Wy, by,
                bq2, bk2, bv2, by2):
    f = np.float32
    bf = mybir.dt.np(BF16)
    WqT = np.ascontiguousarray((Wq.astype(f) / 8.0).T).astype(bf)
    WkT = np.ascontiguousarray(Wk.astype(f).T).astype(bf)
    WvT = np.ascontiguousarray(Wv.astype(f).T).astype(bf)
    WyT = np.ascontiguousarray(Wy.astype(f).T).astype(bf)
    bq_t = np.ascontiguousarray(((bq + bq2).astype(f) / 8.0).reshape(KO, P).T)
    bk_t = np.ascontiguousarray((bk + bk2).astype(f).reshape(KO, P).T)
    bv_t = (bv + bv2).astype(f)[None, :].astype(bf)
    by_t = (by + by2).astype(f)[None, :].astype(bf)

    qT = [np.ascontiguousarray(queries[b].astype(f).T).astype(bf) for b in range(B)]
    kT = [np.ascontiguousarray(keys[b].astype(f).T).astype(bf) for b in range(B)]
    vT = [np.ascontiguousarray(values[b].astype(f).T).astype(bf) for b in range(B)]

    in_maps = []
    for c in range(8):
        b, qi = c // 4, c % 4
        q0 = qi * QS
        ms = mask[b][:, q0:q0 + QS, :]                      # [H, q, k]
        mt = np.ascontiguousarray(ms.transpose(0, 2, 1))    # [H, k, q]
        mb = np.ascontiguousarray(
            mt.reshape(H, NKT, P, QS).transpose(0, 2, 1, 3).reshape(H, P, NKT * QS)
        ).astype(np.int8)
        in_maps.append({
            "qT": np.ascontiguousarray(qT[b][:, q0:q0 + QS]),
            "kT": np.ascontiguousarray(kT[b][:, q0:q0 + QS]),
            "vT": np.ascontiguousarray(vT[b][:, q0:q0 + QS]),
            "maskb": mb,
            "WqT": WqT, "WkT": WkT, "WvT": WvT, "WyT": WyT,
            "bq": bq_t, "bk": bk_t, "bv": bv_t, "by": by_t,
        })
    return in_maps


def kernel(**inputs):
    if "nc" not in _CACHE:
        _CACHE["nc"] = build_program()
    nc = _CACHE["nc"]
    in_maps = prep_inputs(**inputs)
    res = run_bass_kernel_spmd(nc, in_maps, core_ids=list(range(8)))
    out = np.empty((B, S, D), dtype=np.float32)
    for c in range(8):
        b, qi = c // 4, c % 4
        out[b, qi * QS:(qi + 1) * QS, :] = res.results[c]["y"]
    return out


# revision 11
# speedup vs baseline: 1.2692x; 1.2692x over previous
"""MultiHeadAttention Trainium2 Bass kernel.

Model: B=2, S=2048, D_MODEL=1024, H=16 heads, Dh=64.
  q/k/v = x @ W.T + b ; scores = (q k^T)/8 masked-softmax ; out = w @ v ; y = out @ Wy.T + by

Sharding: (batch x sequence) data parallel over 8 cores. Core c handles
batch b = c // 4 and query rows [q0, q0+512) with q0 = (c % 4) * 512.
The K/V projections are token-sharded across the 4 cores of a batch group
(each core projects only its own 512-token chunk) and the results are
AllGathered inside the group, so no projection work is replicated. The
attention and output projection are computed only for the core's query
slice; the final output is a pure concatenation of per-core slices.

Implementation notes:
  - All matmul operands are bf16 (host pre-converts weights/activations);
    PSUM accumulation is fp32. End-to-end max-rel-err ~1.3e-2 vs the 2e-2
    budget. The hardware is bound by the PE instruction stream (~0.7us per
    matmul on this part), so the kernel minimizes matmul count: 768 total.
  - Scores are computed transposed, scoresT[k, q]; heads are processed in
    pairs (2m, 2m+1) on partition halves, the two 64-contraction score
    matmuls issued to disjoint PE row groups via tile_position.
  - Softmax: no max-subtraction (scores are O(6)); exp on ACT straight from
    PSUM, then multiply by the {0,1} int8 mask in-place (alternating
    DVE/GpSimd). The denominator rides as an extra all-ones column
    interleaved in the SBUF-resident V tiles ([V_h(64) | ones(1)] blocks);
    the divide uses reciprocal + GpSimd partition_broadcast.
"""

import numpy as np

import concourse.bass as bass
import concourse.mybir as mybir
import concourse.tile as tile
from concourse import bacc
from concourse.bass_utils import run_bass_kernel_spmd

F32 = mybir.dt.float32
BF16 = mybir.dt.bfloat16
I8 = mybir.dt.int8

B, S, D, H, DH = 2, 2048, 1024, 16, 64
QS = 512          # query rows / projection-token chunk per core
P = 128
KO = D // P       # 8 contraction tiles for the projections
NKT = S // P      # 16 key tiles
E = DH + 1        # V block width incl. ones column
NG = 4            # cores per batch group

_CACHE = {}


def build_program():
    nc = bacc.Bacc("TRN2", target_bir_lowering=False, debug=False, num_devices=8)

    qT = nc.dram_tensor("qT", [D, QS], BF16, kind="ExternalInput")    # queries[b].T q-slice
    kT = nc.dram_tensor("kT", [D, QS], BF16, kind="ExternalInput")    # keys[b].T token chunk
    vT = nc.dram_tensor("vT", [D, QS], BF16, kind="ExternalInput")    # values[b].T token chunk
    maskb = nc.dram_tensor("maskb", [H, P, NKT * QS], I8, kind="ExternalInput")
    WqT = nc.dram_tensor("WqT", [D, D], BF16, kind="ExternalInput")   # (Wq/8).T
    WkT = nc.dram_tensor("WkT", [D, D], BF16, kind="ExternalInput")
    WvT = nc.dram_tensor("WvT", [D, D], BF16, kind="ExternalInput")
    WyT = nc.dram_tensor("WyT", [D, D], BF16, kind="ExternalInput")
    bq = nc.dram_tensor("bq", [P, KO], F32, kind="ExternalInput")     # ((bq+bq2)/8) as [p, m]
    bk = nc.dram_tensor("bk", [P, KO], F32, kind="ExternalInput")
    bv = nc.dram_tensor("bv", [1, D], BF16, kind="ExternalInput")
    by = nc.dram_tensor("by", [1, D], BF16, kind="ExternalInput")
    y = nc.dram_tensor("y", [QS, D], F32, kind="ExternalOutput")

    qT_r = qT.rearrange("(ko p) q -> p ko q", p=P)
    kT_r = kT.rearrange("(ko p) s -> p ko s", p=P)
    vT_r = vT.rearrange("(ko p) s -> p ko s", p=P)
    WqT_r = WqT.rearrange("(ko p) m -> p ko m", p=P)
    WkT_r = WkT.rearrange("(ko p) m -> p ko m", p=P)
    WvT_r = WvT.rearrange("(ko p) m -> p ko m", p=P)
    WyT_r = WyT.rearrange("(ko p) m -> p ko m", p=P)

    def bcast_dram(ap, parts):
        return bass.AP(tensor=ap.tensor, offset=ap.offset, ap=[[0, parts]] + list(ap.ap[1:]))

    IDENT = mybir.ActivationFunctionType.Identity
    EXP = mybir.ActivationFunctionType.Exp
    ADD = mybir.AluOpType.add
    MULT = mybir.AluOpType.mult
    BYPASS = mybir.AluOpType.bypass
    GROUPS = [[0, 1, 2, 3], [4, 5, 6, 7]]

    with tile.TileContext(nc) as tc:
        with (
            tc.tile_pool(name="persist", bufs=1) as persist,
            tc.tile_pool(name="w", bufs=3) as wpool,
            tc.tile_pool(name="kcol", bufs=1) as kcolp,
            tc.tile_pool(name="vtc", bufs=1) as vtcp,
            tc.tile_pool(name="qin", bufs=1) as qinp,
            tc.tile_pool(name="locout", bufs=1) as locoutp,
            tc.tile_pool(name="maskp", bufs=2) as maskp,
            tc.tile_pool(name="eTr", bufs=2) as eTrp,
            tc.tile_pool(name="rec", bufs=2) as recp,
            tc.tile_pool(name="bc", bufs=2) as bcp,
            tc.tile_pool(name="yout", bufs=2) as youtp,
            tc.tile_pool(name="dram", bufs=1, space="DRAM") as dramp,
            tc.tile_pool(name="psA", bufs=2, space="PSUM") as psA,
            tc.tile_pool(name="psS", bufs=2, space="PSUM") as psS,
            tc.tile_pool(name="psT", bufs=2, space="PSUM") as psT,
        ):
            # ---- persistent SBUF ----
            bk_sb = persist.tile([P, KO], F32, tag="bk")
            bq_sb = persist.tile([P, KO], F32, tag="bq")
            bv_bc = persist.tile([P, D], BF16, tag="bv")
            by_bc = persist.tile([P, D], BF16, tag="by")
            nc.sync.dma_start(out=bk_sb, in_=bk[:])
            nc.sync.dma_start(out=bq_sb, in_=bq[:])
            nc.sync.dma_start(out=bv_bc, in_=bcast_dram(bv[:], P))
            nc.sync.dma_start(out=by_bc, in_=bcast_dram(by[:], P))

            KT_m = [persist.tile([P, S], BF16, tag=f"ktm{m}", name=f"KT_m{m}")
                    for m in range(KO)]
            QT_m = [persist.tile([P, QS], BF16, tag=f"qtm{m}", name=f"QT_m{m}")
                    for m in range(KO)]
            VS = [persist.tile([P, H * E], BF16, tag=f"vs{st}", name=f"VS{st}")
                  for st in range(NKT)]
            AT = [persist.tile([P, QS], BF16, tag=f"at{m}", name=f"AT{m}")
                  for m in range(KO)]

            # DRAM bounce buffers for the projection AllGathers
            ktloc = dramp.tile([P, KO, QS], BF16, tag="ktloc", name="ktloc")
            ktgath = dramp.tile([NG, P, KO, QS], BF16, tag="ktgath", name="ktgath")
            vsloc = dramp.tile([P, NG, H * E], BF16, tag="vsloc", name="vsloc")
            vsgath = dramp.tile([NG, P, NG, H * E], BF16, tag="vsgath", name="vsgath")

            wk = wpool.tile([P, KO, D], BF16, tag="w")
            nc.sync.dma_start(out=wk, in_=WkT_r[:])

            # ---- phase K (local chunk): ktl[p, m, q] = Wk @ keysT_chunk + bk ----
            kcol = kcolp.tile([P, KO, QS], BF16, tag="kcol")
            nc.sync.dma_start(out=kcol, in_=kT_r[:])
            ktl = locoutp.tile([P, KO, QS], BF16, tag="loc", name="ktl")
            wq = wpool.tile([P, KO, D], BF16, tag="w")
            nc.sync.dma_start(out=wq, in_=WqT_r[:])
            wv = wpool.tile([P, KO, D], BF16, tag="w")
            nc.sync.dma_start(out=wv, in_=WvT_r[:])
            for m in range(KO):
                ps = psA.tile([P, 512], F32, tag="proj", name="ps")
                for ko in range(KO):
                    nc.tensor.matmul(
                        ps[:], wk[:, ko, m * P:(m + 1) * P], kcol[:, ko, :],
                        start=(ko == 0), stop=(ko == KO - 1))
                nc.scalar.activation(
                    out=ktl[:, m, :], in_=ps[:],
                    func=IDENT, bias=bk_sb[:, m:m + 1], scale=1.0)
            nc.sync.dma_start(out=ktloc, in_=ktl)
            nc.gpsimd.collective_compute(
                "AllGather", BYPASS, replica_groups=GROUPS,
                ins=[ktloc.opt()], outs=[ktgath.opt()])
            for m in range(KO):
                nc.sync.dma_start(
                    out=KT_m[m].rearrange("p (r q) -> p r q", r=NG),
                    in_=ktgath[:, :, m, :].rearrange("r p q -> p r q"))

            # ---- phase V (local chunk) into vsl with interleaved ones ----
            vtc = vtcp.tile([P, KO, QS], BF16, tag="vtc")
            nc.sync.dma_start(out=vtc, in_=vT_r[:])
            vsl = locoutp.tile([P, NG, H * E], BF16, tag="loc", name="vsl")
            vsl_r = vsl.rearrange("p st (h e) -> p st h e", e=E)
            for stl in range(NG):
                for half in range(2):
                    ps = psA.tile([P, 512], F32, tag="proj", name="ps")
                    for ko in range(KO):
                        nc.tensor.matmul(
                            ps[:], vtc[:, ko, stl * P:(stl + 1) * P],
                            wv[:, ko, half * 512:(half + 1) * 512],
                            start=(ko == 0), stop=(ko == KO - 1))
                    nc.vector.tensor_tensor(
                        vsl_r[:, stl, half * 8:(half + 1) * 8, 0:DH],
                        ps.rearrange("p (h d) -> p h d", d=DH),
                        bv_bc[:, half * 512:(half + 1) * 512].rearrange(
                            "p (h d) -> p h d", d=DH),
                        ADD)
            nc.vector.memset(vsl_r[:, :, :, DH:E], 1.0)
            nc.sync.dma_start(out=vsloc, in_=vsl)
            nc.gpsimd.collective_compute(
                "AllGather", BYPASS, replica_groups=GROUPS,
                ins=[vsloc.opt()], outs=[vsgath.opt()])
            for st in range(NKT):
                nc.sync.dma_start(
                    out=VS[st], in_=vsgath[st // NG, :, st % NG, :])

            # ---- phase Q: QT_m[m][dout_p, q] = (Wq/8) @ queries[b].T + bq/8 ----
            qin = qinp.tile([P, KO, QS], BF16, tag="qin")
            nc.sync.dma_start(out=qin, in_=qT_r[:])
            for m in range(KO):
                ps = psA.tile([P, 512], F32, tag="proj", name="ps")
                for ko in range(KO):
                    nc.tensor.matmul(
                        ps[:], wq[:, ko, m * P:(m + 1) * P], qin[:, ko, :],
                        start=(ko == 0), stop=(ko == KO - 1))
                nc.scalar.activation(
                    out=QT_m[m], in_=ps[:],
                    func=IDENT, bias=bq_sb[:, m:m + 1], scale=1.0)

            # ---- attention pair hm: heads (2hm, 2hm+1) on partition halves ----
            def attn_pair(hm):
                hA, hB = 2 * hm, 2 * hm + 1
                mks = []
                for mh in range(2):
                    mk = maskp.tile([P, 2, NKT * QS // 2], I8, tag="mask", name="mk")
                    nc.sync.dma_start(
                        out=mk[:, 0, :],
                        in_=maskb[hA, :, mh * 4096:(mh + 1) * 4096])
                    nc.sync.dma_start(
                        out=mk[:, 1, :],
                        in_=maskb[hB, :, mh * 4096:(mh + 1) * 4096])
                    mks.append(mk)
                pattA = psT.tile([P, QS], F32, tag="patt", name="pattA")
                pattB = psT.tile([P, QS], F32, tag="patt", name="pattB")
                for kt in range(NKT):
                    pscr = psS.tile([P, 2 * QS], F32, tag="scores", name="pscr")
                    nc.tensor.matmul(
                        pscr[:, 0:QS], KT_m[hm][0:DH, kt * P:(kt + 1) * P],
                        QT_m[hm][0:DH, :], start=True, stop=True,
                        tile_position=(0, 0))
                    nc.tensor.matmul(
                        pscr[:, QS:2 * QS], KT_m[hm][DH:P, kt * P:(kt + 1) * P],
                        QT_m[hm][DH:P, :], start=True, stop=True,
                        tile_position=(64, 0))
                    eT = eTrp.tile([P, 2 * QS], BF16, tag="eTr", name="eT")
                    nc.scalar.activation(out=eT[:], in_=pscr[:], func=EXP)
                    mslice = mks[kt // 8][:, :, (kt % 8) * QS:(kt % 8 + 1) * QS]
                    ev = eT.rearrange("p (h q) -> p h q", q=QS)
                    eng = nc.vector if kt % 2 == 0 else nc.gpsimd
                    eng.tensor_tensor(ev[:], ev[:], mslice, MULT)
                    vs_r = VS[kt].rearrange("p (h e) -> p h e", e=E)
                    nc.tensor.matmul(
                        pattA[0:E, :], vs_r[:, hA, :], eT[:, 0:QS],
                        start=(kt == 0), stop=(kt == NKT - 1))
                    nc.tensor.matmul(
                        pattB[0:E, :], vs_r[:, hB, :], eT[:, QS:2 * QS],
                        start=(kt == 0), stop=(kt == NKT - 1))
                for hp, patt in ((0, pattA), (1, pattB)):
                    rec = recp.tile([1, QS], F32, tag="rec", name="rec")
                    nc.vector.reciprocal(out=rec[:], in_=patt[DH:E, :])
                    bc = bcp.tile([DH, QS], F32, tag="bc", name="bc")
                    nc.gpsimd.partition_broadcast(bc[:], rec[:])
                    nc.vector.tensor_tensor(
                        AT[hm][hp * DH:(hp + 1) * DH, :], patt[0:DH, :], bc[:],
                        MULT)

            for hm in range(KO):
                attn_pair(hm)

            # ---- phase Y: y = merged @ Wy.T + by ----
            wy = wpool.tile([P, KO, D], BF16, tag="w")
            nc.sync.dma_start(out=wy, in_=WyT_r[:])
            for qt in range(4):
                for half in range(2):
                    ps = psA.tile([P, 512], F32, tag="proj", name="ps")
                    for hm in range(KO):
                        nc.tensor.matmul(
                            ps[:], AT[hm][:, qt * P:(qt + 1) * P],
                            wy[:, hm, half * 512:(half + 1) * 512],
                            start=(hm == 0), stop=(hm == KO - 1))
                    yo = youtp.tile([P, 512], F32, tag="yo", name="yo")
                    nc.vector.tensor_tensor(
                        yo[:], ps[:], by_bc[:, half * 512:(half + 1) * 512], ADD)
                    nc.sync.dma_start(
                        out=y[qt * P:(qt + 1) * P, half * 512:(half + 1) * 512],
                        in_=yo[:])

    nc.compile()
    return nc


def prep_inputs(queries, keys, values, mask, Wq, bq, Wk, bk, Wv, bv, Wy, by,
                bq2, bk2, bv2, by2):
    f = np.float32
    bf = mybir.dt.np(BF16)
    WqT = np.ascontiguousarray((Wq.astype(f) / 8.0).T).astype(bf)
    WkT = np.ascontiguousarray(Wk.astype(f).T).astype(bf)
    WvT = np.ascontiguousarray(Wv.astype(f).T).astype(bf)
    WyT = np.ascontiguousarray(Wy.astype(f).T).astype(bf)
    bq_t = np.ascontiguousarray(((bq + bq2).astype(f) / 8.0).reshape(KO, P).T)
    bk_t = np.ascontiguousarray((bk + bk2).astype(f).reshape(KO, P).T)
    bv_t = (bv + bv2).astype(f)[None, :].astype(bf)
    by_t = (by + by2).astype(f)[None, :].astype(bf)

    qT = [np.ascontiguousarray(queries[b].astype(f).T).astype(bf) for b in range(B)]
    kT = [np.ascontiguousarray(keys[b].astype(f).T).astype(bf) for b in range(B)]
    vT = [np.ascontiguousarray(values[b].astype(f).T).astype(bf) for b in range(B)]

    in_maps = []
    for c in range(8):
        b, qi = c // 4, c % 4
        q0 = qi * QS
        ms = mask[b][:, q0:q0 + QS, :]                      # [H, q, k]
        mt = np.ascontiguousarray(ms.transpose(0, 2, 1))    # [H, k, q]
        mb = np.ascontiguousarray(
            mt.reshape(H, NKT, P, QS).transpose(0, 2, 1, 3).reshape(H, P, NKT * QS)
        ).astype(np.int8)
        in_maps.append({
            "qT": np.ascontiguousarray(qT[b][:, q0:q0 + QS]),
            "kT": np.ascontiguousarray(kT[b][:, q0:q0 + QS]),
            "vT": np.ascontiguousarray(vT[b][:, q0:q0 + QS]),
            "maskb": mb,
            "WqT": WqT, "WkT": WkT, "WvT": WvT, "WyT": WyT,
            "bq": bq_t, "bk": bk_t, "bv": bv_t, "by": by_t,
        })
    return in_maps


def kernel(**inputs):
    if "nc" not in _CACHE:
        _CACHE["nc"] = build_program()
    nc = _CACHE["nc"]
    in_maps = prep_inputs(**inputs)
    res = run_bass_kernel_spmd(nc, in_maps, core_ids=list(range(8)))
    out = np.empty((B, S, D), dtype=np.float32)
    for c in range(8):
        b, qi = c // 4, c % 4
        out[b, qi * QS:(qi + 1) * QS, :] = res.results[c]["y"]
    return out
